# revision 3
# baseline (speedup 1.0000x reference)
"""TRN2 Bass kernel for nn_CustomLinear_66005057405513.

Computes y = FFT_4096(w * x)[:, :3072] for x: [4096, 4096] complex64
(given as interleaved float pairs) and w: [4096] complex64 twiddles.

Strategy: data-parallel over 8 NeuronCores (512 batch rows each). On each
core, a two-stage radix-64 FFT with all twiddles folded into precomputed
matrices:

  n = 64*o + i, k = p + 64*q, q < 48:
    stage 1 (per i):  A[b, i, p] = sum_o C1[i][o, p] * x[b, 64o+i]
                      C1[i][o, p] = W64^(op) * w[64o+i]
    stage 2 (per p):  y[b, p+64q] = sum_i C2[p][i, q] * A[b, i, p]
                      C2[p][i, q] = W4096^(ip) * W64^(iq)

Complex values ride as interleaved (re, im) float pairs; each complex
matmul is one real matmul with the pair-encoded matrix (contraction
K = 128 = full PE partition dim), fp16 compute with fp32 PSUM accum.

Layout/scheduling choices (all verified by A/B timing):
- x is pre-transposed on the host to [oc, (slab, i, b)] so stage-1 lhsT
  slices come straight out of SBUF: no stage-1 PE transposes, no extra
  PSUM round-trips. Only the structurally-required stage-2 flip of A
  (b <-> ic) runs on the PE.
- Software pipeline at group granularity: stage-2 of slab s-1 is
  interleaved with stage-1 of slab s so the in-order ACT/DVE queues
  never head-of-line block on a far-away dependency.
- y SBUF layout is p-major so PSUM evacuations write contiguous runs
  and each column half can stream out as soon as its 8 evacs land.
- PSUM evacuations statically split across ACT/DVE by dtype (DVE gets
  the 2-byte copies it can run at 2x).
"""

import numpy as np

import concourse.bass as bass
import concourse.mybir as mybir
from concourse import bacc
from concourse.tile import TileContext
from concourse.masks import make_identity
from concourse.bass_utils import run_bass_kernel_spmd

O = I = 64
N_FFT = O * I          # 4096
Q = 48                 # q < 48  <=>  k < 3072
OUT_F = Q * O * 2      # 6144 halfs per output row
B_TOTAL = 4096
N_CORES = 8
B_LOCAL = B_TOTAL // N_CORES  # 512
SLABS = B_LOCAL // 128        # 4


def _make_tables(w_complex):
    oo = np.arange(O)
    W64 = np.exp(-2j * np.pi * np.outer(oo, oo) / O)
    WN = np.exp(-2j * np.pi * np.outer(np.arange(I), oo) / N_FFT)

    def pairmat(C):
        K, M = C.shape
        G = np.empty((2 * K, 2 * M), np.float64)
        G[0::2, 0::2] = C.real
        G[1::2, 0::2] = -C.imag
        G[0::2, 1::2] = C.imag
        G[1::2, 1::2] = C.real
        return G

    g1 = np.empty((128, I, 128), np.float64)
    for i in range(I):
        C1 = W64 * w_complex[64 * oo + i][:, None]
        g1[:, i, :] = pairmat(C1)
    g2 = np.empty((128, O, 96), np.float64)
    for p in range(O):
        C2 = WN[:, p][:, None] * W64[:, :Q]
        g2[:, p, :] = pairmat(C2)
    return g1, g2


def _build_nc(reps=1):
    f32 = mybir.dt.float32
    f16 = mybir.dt.float16

    nc = bacc.Bacc(None, target_bir_lowering=False, debug=False)
    # x host layout: [oc, (slab, i, b)] -- already transposed for stage 1
    x = nc.declare_dram_parameter("x", [128, SLABS * 8192], f16, isOutput=False)
    w1 = nc.declare_dram_parameter("w1", [128, I * 128], f16, isOutput=False)
    w2 = nc.declare_dram_parameter("w2", [128, O * 96], f16, isOutput=False)
    # y DRAM layout: [b, (p, q, c)] -- host untangles to k = p + 64q
    y = nc.declare_dram_parameter("y", [B_LOCAL, OUT_F], f16, isOutput=True)

    yc = [0]

    def copy_y(out_ap, in_ap):
        # 6 of every 16 y-evacs on ACT, 10 on DVE (measured balance)
        k = yc[0] % 8
        yc[0] += 1
        if k in (0, 3, 6):
            nc.scalar.copy(out_ap, in_ap)
        else:
            nc.vector.tensor_copy(out_ap, in_ap)

    with TileContext(nc) as tc:
        with (
            tc.tile_pool(name="const", bufs=1) as cpool,
            tc.tile_pool(name="xp", bufs=2) as xpool,
            tc.tile_pool(name="ap", bufs=2) as apool,
            tc.tile_pool(name="yp", bufs=2) as ypool,
            tc.tile_pool(name="ts", bufs=3) as tspool,
            tc.tile_pool(name="pm1", bufs=2, space="PSUM") as pm1,
            tc.tile_pool(name="pt2", bufs=2, space="PSUM") as pt2,
            tc.tile_pool(name="pm2", bufs=2, space="PSUM") as pm2,
        ):
            ident = cpool.tile([128, 128], f16, name="ident")
            make_identity(nc, ident[:])
            w1s = cpool.tile([128, I * 128], f16, name="w1s")
            nc.scalar.dma_start(out=w1s[:], in_=w1[:])
            w2s = cpool.tile([128, O * 96], f16, name="w2s")
            nc.scalar.dma_start(out=w2s[:], in_=w2[:])
            w1v = w1s[:].rearrange("k (i n) -> k i n", i=I)
            w2v = w2s[:].rearrange("k (p n) -> k p n", p=O)

            def job(_iv=None):
                live = {}

                def s1_group(g):
                    xv, av = live["xv"], live["av"]
                    m1 = pm1.tile([128, 1024], f32, name="m1")
                    for j in range(8):
                        i = g * 8 + j
                        nc.tensor.matmul(
                            m1[:, j * 128:(j + 1) * 128],
                            lhsT=xv[:, i, :], rhs=w1v[:, i, :],
                            start=True, stop=True)
                    nc.scalar.copy(
                        av[:, g * 8:(g + 1) * 8, :, :],
                        m1[:].rearrange("b (i p c) -> b i p c", i=8, c=2))

                def s2_group(g):
                    Ab, yv = live["Ab_prev"], live["yv"]
                    t2p = pt2.tile([128, 1024], f16, name="t2p")
                    for j in range(8):
                        p = g * 8 + j
                        nc.tensor.transpose(
                            t2p[:, j * 128:(j + 1) * 128],
                            Ab[:, p * 128:(p + 1) * 128], ident[:])
                    t2s = tspool.tile([128, 1024], f16, name="t2s")
                    nc.vector.tensor_copy(t2s[:], t2p[:])
                    for h in range(2):
                        m2 = pm2.tile([128, 512], f32, name="m2")[:, :4 * 96]
                        for j in range(4):
                            jj = h * 4 + j
                            p = g * 8 + jj
                            nc.tensor.matmul(
                                m2[:, j * 96:(j + 1) * 96],
                                lhsT=t2s[:, jj * 128:(jj + 1) * 128],
                                rhs=w2v[:, p, :], start=True, stop=True)
                        p0 = g * 8 + h * 4
                        copy_y(yv[:, p0:p0 + 4, :, :],
                               m2[:].rearrange("b (p q c) -> b p q c", q=Q, c=2))

                for s in range(SLABS + 1):
                    if s < SLABS:
                        xs = xpool.tile([128, 8192], f16, name="xs")
                        for h in range(2):
                            nc.sync.dma_start(
                                out=xs[:, h * 4096:(h + 1) * 4096],
                                in_=x[:, s * 8192 + h * 4096:
                                      s * 8192 + (h + 1) * 4096])
                        live["xv"] = xs[:].rearrange("k (i b) -> k i b", i=I)
                        Ab = apool.tile([128, 8192], f16, name="Ab")
                        live["av"] = Ab[:].rearrange(
                            "b (p i c) -> b i p c", p=O, c=2)
                    if s > 0:
                        Yb = ypool.tile([128, OUT_F], f16, name="Yb")
                        # p-major: col = p*96 + 2q + c
                        live["yv"] = Yb[:].rearrange(
                            "b (p q c) -> b p q c", p=O, c=2)
                    for g in range(8):
                        if s > 0:
                            s2_group(g)
                            if g == 3 or g == 7:
                                hh = g // 4
                                nc.scalar.dma_start(
                                    out=y[(s - 1) * 128:s * 128,
                                          hh * 3072:(hh + 1) * 3072],
                                    in_=Yb[:, hh * 3072:(hh + 1) * 3072])
                        if s < SLABS:
                            s1_group(g)
                    if s < SLABS:
                        live["Ab_prev"] = Ab

            if reps > 1:
                with tc.For_i(0, reps, 1) as _i:
                    job(_i)
            else:
                job()

    nc.compile()
    return nc


_NC_CACHE = {}


def _get_nc():
    if "nc" not in _NC_CACHE:
        _NC_CACHE["nc"] = _build_nc()
    return _NC_CACHE["nc"]


def _host_inputs(x_real, weights_real):
    wr = np.asarray(weights_real, dtype=np.float64)
    wc = wr[0::2] + 1j * wr[1::2]
    g1, g2 = _make_tables(wc)
    w1 = np.ascontiguousarray(g1.reshape(128, -1)).astype(np.float16)
    w2 = np.ascontiguousarray(g2.reshape(128, -1)).astype(np.float16)
    x = np.asarray(x_real)
    B = x.shape[0]
    bl = B // N_CORES
    xh = x.reshape(B, O, I, 2).astype(np.float16)  # [b, o, i, c]
    maps = []
    for c in range(N_CORES):
        xc = xh[c * bl:(c + 1) * bl].reshape(SLABS, 128, O, I, 2)
        # [bt, b, o, i, c] -> [o, c, bt, i, b] = [oc, (slab, i, b)]
        xf = np.ascontiguousarray(xc.transpose(2, 4, 0, 3, 1)).reshape(128, -1)
        maps.append({"x": xf, "w1": w1, "w2": w2})
    return maps


def kernel(x_real, weights_real):
    nc = _get_nc()
    in_maps = _host_inputs(x_real, weights_real)
    res = run_bass_kernel_spmd(nc, in_maps, list(range(N_CORES)))
    outs = []
    for c in range(N_CORES):
        v = np.asarray(res.results[c]["y"], dtype=np.float32)
        # device col = p*96 + 2q + c  ->  k = p + 64q
        v = v.reshape(B_LOCAL, O, Q, 2).transpose(0, 2, 1, 3)
        v = v.reshape(B_LOCAL, Q * O, 2)
        outs.append((v[..., 0] + 1j * v[..., 1]).astype(np.complex64))
    return np.concatenate(outs, axis=0)
